# revision 27
# baseline (speedup 1.0000x reference)
"""BipartiteGCN Trainium2 kernel v5 (8 NeuronCores, Bass/Tile).

v3 architecture (256B fp16 rows, chunked tables + chunked AllGathers,
width-512 one-hot aggregation, layer-1 a2p/co supervision filtering,
joint p2a gather) plus:
  - asymmetric source chunks (4096/4096/3584/724 rows): the last AllGather
    of every phase is tiny, shortening the phase-boundary serial tail.
  - the layer-1 p2a stage-B pass (a2a) fused into co0's stage-B per block,
    so the a2a AllGathers fire spread across co0 instead of bunched after.
  - supervision via AllToAll: p2 rows for sup pairs gathered locally on
    the paper-owner core, AllToAll'ed to the author-owner core (no p2
    AllGathers; recv side is plain contiguous loads).
  - scalar-engine psum evacuation, fused multiply+reduce for sup dots,
    deeper gather/one-hot pools.
"""

import numpy as np

import concourse.bacc as bacc
import concourse.mybir as mybir
import concourse.tile as tile
from concourse.bass_utils import run_bass_kernel_spmd

F32 = mybir.dt.float32
F16 = mybir.dt.float16
I16 = mybir.dt.int16

NCORES = 8
D = 128
N_AUTHOR = 100000
N_PAPER_ACT = 100000
SHARD = N_AUTHOR // NCORES      # 12500
SB = 512
NSB = (SHARD + SB - 1) // SB    # 25
MACRO = 5
MACRO_J = 2                     # joint phase: 2 PSUM banks per sb
NCHUNK = 4
CH_START = [0, 4096, 8192, 11776]
CH_SIZE = [4096, 4096, 3584, SHARD - 11776]
CH_SB = [(0, 8), (8, 16), (16, 23), (23, 25)]
PAD_DST = 5000.0
N_LAYERS = 2
SUP_CHUNK = 8


def _sb_width(sb):
    return min(SB, SHARD - sb * SB)


def _chunk_of_local(r):
    return np.searchsorted(np.array(CH_START[1:] + [SHARD]), r, side="right")


def _chunk_row(node):
    k = node // SHARD
    r = node - k * SHARD
    c = _chunk_of_local(r)
    sz = np.array(CH_SIZE)[c]
    st = np.array(CH_START)[c]
    return c, k * sz + (r - st)


# ---------------------------------------------------------------- host prep

def _wrap_idx(idx):
    n = len(idx)
    w = np.zeros((128, n // 16), np.int16)
    base = idx.astype(np.int16).reshape(n // 16, 16).T
    for g in range(8):
        w[16 * g:16 * g + 16, :] = base
    return w


def _build_tiles_one_core(src, dst_local):
    sb_id = dst_local // SB
    off = dst_local - sb_id * SB
    c, crow = _chunk_row(src)
    tiles = {}
    nt = np.zeros((NSB, NCHUNK), np.int64)
    order = np.lexsort((off, c, sb_id))
    sb_s, c_s = sb_id[order], c[order]
    off_s, crow_s = off[order], crow[order]
    key = sb_s * NCHUNK + c_s
    bounds = np.flatnonzero(np.diff(key)) + 1
    starts = np.concatenate(([0], bounds))
    ends = np.concatenate((bounds, [len(key)]))
    for s, e in zip(starts, ends):
        sb, cc = int(sb_s[s]), int(c_s[s])
        r = crow_s[s:e]
        o = off_s[s:e]
        group = [(r[i:i + 128], o[i:i + 128]) for i in range(0, e - s, 128)]
        tiles[(sb, cc)] = group
        nt[sb, cc] = len(group)
    return nt, tiles


def _emit_direction(all_tiles, global_nt, macro):
    nmacro = (NSB + macro - 1) // macro
    idx_stream = []
    dstloc_cols = []
    meta = []
    first_seen = set()
    last_pos = {}
    for sb in range(NSB):
        tot = int(global_nt[sb].sum())
        assert tot > 0
        cnt = 0
        for c in range(NCHUNK):
            for t in range(int(global_nt[sb, c])):
                cnt += 1
                if cnt == tot:
                    last_pos[sb] = (c, t)
    for m in range(nmacro):
        sbs = range(m * macro, min((m + 1) * macro, NSB))
        for c in range(NCHUNK):
            tl = []
            for sb in sbs:
                group = all_tiles.get((sb, c), [])
                for t in range(int(global_nt[sb, c])):
                    if t < len(group):
                        r, o = group[t]
                    else:
                        r = np.zeros(0, np.int64)
                        o = np.zeros(0, np.int64)
                    n = len(r)
                    first = sb not in first_seen
                    if first:
                        first_seen.add(sb)
                    last = last_pos[sb] == (c, t)
                    src128 = np.zeros(128, np.int64)
                    dl128 = np.full(128, PAD_DST, np.float32)
                    src128[:n] = r
                    dl128[:n] = o[:n]
                    idx_stream.append(src128)
                    dstloc_cols.append(dl128)
                    tl.append((sb, first, last))
            meta.append(tl)
    ntiles = len(idx_stream)
    idx_flat = np.concatenate(idx_stream) if ntiles else np.zeros(0, np.int64)
    dstloc = (np.stack(dstloc_cols, axis=1) if ntiles
              else np.zeros((128, 0), np.float32))
    return _wrap_idx(idx_flat), dstloc.astype(np.float32), meta


def _prep_direction(src_all, dst_all, macro=MACRO, ncores=NCORES):
    owner = dst_all // SHARD
    per_core = []
    nts = []
    for k in range(ncores):
        m = owner == k
        nt, tiles = _build_tiles_one_core(src_all[m], dst_all[m] - k * SHARD)
        nts.append(nt)
        per_core.append(tiles)
    global_nt = np.maximum.reduce(nts)
    global_nt[:, 0] = np.maximum(global_nt[:, 0], 1)
    idxs, dstlocs, metas = [], [], []
    for k in range(ncores):
        iw, dl, meta = _emit_direction(per_core[k], global_nt, macro)
        idxs.append(iw)
        dstlocs.append(dl)
        metas.append(meta)
    degs = []
    for k in range(ncores):
        m = owner == k
        deg = np.bincount(dst_all[m] - k * SHARD, minlength=SHARD)
        degs.append(np.stack([deg, np.ones(SHARD)]).astype(np.float16))
    return idxs, dstlocs, degs, metas[0]


def _prep_sup(sup_a, sup_p, ncores=NCORES):
    """A2A layout: paper-owner kp gathers p2 rows for its pairs grouped by
    author-owner ka; AllToAll ships block (kp -> ka); author-owner computes
    products with locally gathered a2 rows in the same pair order."""
    kp = sup_p // SHARD
    ka = sup_a // SHARD
    a_loc = sup_a - ka * SHARD
    p_loc = sup_p - kp * SHARD
    cnt = np.zeros((ncores, ncores), np.int64)
    np.add.at(cnt, (kp, ka), 1)
    blk_t = int((cnt.max() + 127) // 128)
    blk = blk_t * 128
    p_send = np.zeros((ncores, ncores * blk), np.int64)
    a_recv = np.zeros((ncores, ncores * blk), np.int64)
    pos = [[None] * ncores for _ in range(ncores)]
    for s in range(ncores):
        for r in range(ncores):
            m = np.flatnonzero((kp == s) & (ka == r))
            sub = m[np.lexsort((m, p_loc[m], a_loc[m]))]
            nb = len(sub)
            p_send[s, r * blk:r * blk + nb] = p_loc[sub]
            a_recv[r, s * blk:s * blk + nb] = a_loc[sub]
            pos[r][s] = sub
    packs = []
    for k in range(ncores):
        packs.append((_wrap_idx(a_recv[k]), _wrap_idx(p_send[k]), pos[k]))
    return packs, blk_t


# ------------------------------------------------------------- program build

PHASES = ["a2p0", "p2a0", "co0", "a2p1", "co1"]


def _build_program(meta, reps=1):
    nc = bacc.Bacc("TRN2", target_bir_lowering=False, debug=False,
                   enable_asserts=False, num_devices=NCORES,
                   num_swdge_queues=4)

    def din(name, shape, dt=F16):
        return nc.dram_tensor(name, shape, dt, kind="ExternalInput").ap()

    author_c = [din(f"author_c{c}", [8 * CH_SIZE[c], D]) for c in range(NCHUNK)]
    # own interleaved paper chunks: [:, :128] = own paper_t0, [:, 128:] = 0
    pint_in = [din(f"pint_c{c}", [CH_SIZE[c], 2 * D]) for c in range(NCHUNK)]
    xaT0 = din("xaT0", [128, SHARD])
    xpT0 = din("xpT0", [128, SHARD])
    w_cat = din("w_cat", [128, 128 * 10])
    bias_cat = din("bias_cat", [2, 128 * 6])
    iota_in = din("iota512", [128, 512], F16)
    ident_in = din("identity", [128, 128])
    idx_in, dl_in, deg_in = {}, {}, {}
    for ph in PHASES:
        ntp = meta[f"ntiles_{ph}"]
        idx_in[ph] = din(f"idx_{ph}", [128, ntp * 8], I16)
        dl_in[ph] = din(f"dl_{ph}", [128, ntp], F32)
        deg_in[ph] = din(f"deg_{ph}", [2, SHARD])
    blk_t = int(meta["sup_blk_t"])
    nsup = NCORES * blk_t * 128
    nsupt = nsup // 128                      # total product tiles
    idx_sup_a = din("idx_sup_a", [128, nsup // 16], I16)
    idx_sup_p = din("idx_sup_p", [128, nsup // 16], I16)
    out_sup = nc.dram_tensor("out_sup", [128, nsupt], F32,
                             kind="ExternalOutput").ap()

    # sizing: G pool bytes and idx columns per macro
    g_bytes_max = 1
    idx_cols_max = 16
    for ph in PHASES:
        mm = meta[ph]
        nmacro = len(mm) // NCHUNK
        elem = 256 if ph == "p2a0" else 128
        for tl in mm:
            g_bytes_max = max(g_bytes_max, len(tl) * elem * 2)
        for m in range(nmacro):
            cols = sum(len(mm[m * NCHUNK + c]) for c in range(NCHUNK)) * 8
            idx_cols_max = max(idx_cols_max, cols)
    g_cols = g_bytes_max // 2   # fp16 elems per partition

    with tile.TileContext(nc) as tc:
        with tc.tile_pool(name="persist", bufs=1) as pp, \
             tc.tile_pool(name="gat", bufs=6) as gp, \
             tc.tile_pool(name="oneh", bufs=10) as sp, \
             tc.tile_pool(name="stageb", bufs=3) as bp, \
             tc.tile_pool(name="degp", bufs=4) as dgp, \
             tc.tile_pool(name="idxp", bufs=3) as ixp, \
             tc.tile_pool(name="supp", bufs=3) as sup_pl, \
             tc.tile_pool(name="stg", bufs=1) as stg, \
             tc.tile_pool(name="psA", bufs=5, space="PSUM") as psA, \
             tc.tile_pool(name="psB", bufs=2, space="PSUM") as psB, \
             tc.tile_pool(name="psT", bufs=1, space="PSUM") as psT, \
             tc.tile_pool(name="dram", bufs=1, space="DRAM") as drp:

            xaT = pp.tile([128, SHARD], F16, name="xaT")
            xpT = pp.tile([128, SHARD], F16, name="xpT")
            agg_h = pp.tile([128, SHARD], F16, name="agg_h")
            iota = pp.tile([128, 512], F16, name="iota")
            ident = pp.tile([128, 128], F16, name="ident")
            w_t = pp.tile([128, 128 * 10], F16, name="w_t")
            bias_t = pp.tile([2, 128 * 6], F16, name="bias_t")
            dl_t = {p: pp.tile([128, meta[f"ntiles_{p}"]], F32, name=f"dl_{p}")
                    for p in PHASES}
            out_sb = pp.tile([128, nsupt], F32, name="out_sb")

            nc.sync.dma_start(out=xaT[:], in_=xaT0[:])
            nc.sync.dma_start(out=xpT[:], in_=xpT0[:])
            nc.sync.dma_start(out=iota[:], in_=iota_in[:])
            nc.sync.dma_start(out=ident[:], in_=ident_in[:])
            nc.sync.dma_start(out=w_t[:], in_=w_cat[:])
            nc.sync.dma_start(out=bias_t[:], in_=bias_cat[:])
            for p in PHASES:
                nc.sync.dma_start(out=dl_t[p][:], in_=dl_in[p][:])
            idx_sup_a_t = pp.tile([128, nsup // 16], I16, name="supa")
            idx_sup_p_t = pp.tile([128, nsup // 16], I16, name="supb")
            nc.sync.dma_start(out=idx_sup_a_t[:], in_=idx_sup_a[:])
            nc.sync.dma_start(out=idx_sup_p_t[:], in_=idx_sup_p[:])

            def own_chunks(name, width=D):
                return [drp.tile([CH_SIZE[c], width], F16, name=f"{name}_o{c}")
                        for c in range(NCHUNK)]

            def full_chunks(name, width=D):
                return [drp.tile([8 * CH_SIZE[c], width], F16,
                                 addr_space="Shared", name=f"{name}_f{c}")
                        for c in range(NCHUNK)]

            def w_slice(l, slot):
                o = (l * 5 + slot) * 128
                return w_t[:, o:o + 128]

            def bias_slice(l, ph):
                o = (l * 3 + ph) * 128
                return bias_t[:, o:o + 128]

            def transpose_writeback(xown, sb, wdt, own_out):
                pt = psT.tile([128, 512], F16, tag="tr", name="pt")
                nch = (wdt + 127) // 128
                for j in range(nch):
                    cw = min(128, wdt - j * 128)
                    nc.tensor.matmul(
                        out=pt[:cw, j * 128:j * 128 + 128],
                        lhsT=xown[:, sb * SB + j * 128:sb * SB + j * 128 + cw],
                        rhs=ident[:], is_transpose=True,
                        start=(j == 0), stop=(j == nch - 1))
                rm = bp.tile([128, 512], F16, tag="rm", name="rm")
                nc.scalar.copy(out=rm[:, :nch * 128], in_=pt[:, :nch * 128])
                cid = int(_chunk_of_local(sb * SB))
                row0 = sb * SB - CH_START[cid]
                for j in range(nch):
                    cw = min(128, wdt - j * 128)
                    nc.sync.dma_start(
                        out=own_out[cid][row0 + j * 128:row0 + j * 128 + cw,
                                         0:128],
                        in_=rm[:cw, j * 128:j * 128 + 128])
                return cid

            def fire_ag(cid, own_out, ag, ag_src):
                src = ag_src[cid] if ag_src is not None else own_out[cid]
                nc.gpsimd.collective_compute(
                    "AllGather", mybir.AluOpType.bypass,
                    replica_groups=[list(range(NCORES))],
                    ins=[src[:]], outs=[ag[cid][:]])

            def stage_b(ph_deg, sb, pa, xown, wdir, wself, biasp, own_out,
                        ag, ag_src, co_mode, wdt=None):
                """Shared stage-B: transform+bias, xown update, transpose,
                row-major writeback, chunk AG firing."""
                wdt = wdt or _sb_width(sb)
                agg_sb = bp.tile([128, 512], F16, tag="aggsb", name="aggsb")
                nc.scalar.copy(out=agg_sb[:, :wdt], in_=pa[:, :wdt])
                deg_t = dgp.tile([2, 512], F16, tag="deg", name="degt")
                nc.sync.dma_start(
                    out=deg_t[:, :wdt],
                    in_=deg_in[ph_deg][:, sb * SB:sb * SB + wdt])
                pb = psB.tile([128, 512], F32, tag="out", name="pb")
                nc.tensor.matmul(out=pb[:, :wdt], lhsT=wdir,
                                 rhs=agg_sb[:, :wdt], start=True, stop=False)
                if not co_mode:
                    nc.tensor.matmul(
                        out=pb[:, :wdt], lhsT=wself,
                        rhs=xown[:, sb * SB:sb * SB + wdt],
                        start=False, stop=False)
                nc.tensor.matmul(out=pb[:, :wdt], lhsT=biasp,
                                 rhs=deg_t[:2, :wdt], start=False, stop=True)
                if co_mode:
                    nc.vector.tensor_tensor(
                        out=xown[:, sb * SB:sb * SB + wdt],
                        in0=pb[:, :wdt],
                        in1=xown[:, sb * SB:sb * SB + wdt],
                        op=mybir.AluOpType.add)
                else:
                    nc.scalar.copy(
                        out=xown[:, sb * SB:sb * SB + wdt], in_=pb[:, :wdt])
                cid = transpose_writeback(xown, sb, wdt, own_out)
                if ag is not None and sb == CH_SB[cid][1] - 1:
                    fire_ag(cid, own_out, ag, ag_src)

            def process_phase(ph, src_tbls, xown, wdir, wself, biasp,
                              own_out, ag=None, ag_src=None, co_mode=False,
                              joint=None, post_sb=None):
                """joint: None, or (agg_hold_tile,) for the p2a joint phase
                (elem=256, second matmul per tile into a held aggregate).
                post_sb: optional callback(sb, wdt) run after stage_b(sb)."""
                mm = meta[ph]
                nmacro = len(mm) // NCHUNK
                macro = (NSB + nmacro - 1) // nmacro
                elem = 256 if joint else 128
                tile_col = 0
                psum0, psum1 = {}, {}
                left_of_sb = {sb: 0 for sb in range(NSB)}
                for tl in mm:
                    for (sb, _f, _l) in tl:
                        left_of_sb[sb] += 1
                col_off = 0
                for m in range(nmacro):
                    cols = sum(len(mm[m * NCHUNK + c])
                               for c in range(NCHUNK)) * 8
                    if cols == 0:
                        continue
                    idx_t = ixp.tile([128, idx_cols_max], I16, tag="idx",
                                     name="idxt")
                    nc.sync.dma_start(
                        out=idx_t[:, :cols],
                        in_=idx_in[ph][:, col_off:col_off + cols])
                    mac_off = 0
                    for c in range(NCHUNK):
                        tl = mm[m * NCHUNK + c]
                        ntl = len(tl)
                        if ntl == 0:
                            continue
                        nidx = ntl * 128
                        G = gp.tile([128, g_cols], F16, tag="G", name="G")
                        nc.gpsimd.dma_gather(
                            G[:, :ntl * elem].rearrange(
                                "p (c e) -> p c e", e=elem),
                            src_tbls[c][:, :],
                            idx_t[:, mac_off:mac_off + ntl * 8],
                            nidx, nidx, elem,
                            single_packet=(nidx <= 1024), queue_num=c)
                        mac_off += ntl * 8
                        for ti, (sb, first, _last) in enumerate(tl):
                            if sb not in psum0:
                                psum0[sb] = psA.tile([128, 512], F32,
                                                     tag="agg", name="agg")
                                if joint:
                                    psum1[sb] = psA.tile(
                                        [128, 512], F32, tag="agg",
                                        name="agg")
                            pa = psum0[sb]
                            S = sp.tile([128, 512], F16, tag="S", name="S")
                            nc.vector.tensor_scalar(
                                out=S[:], in0=iota[:],
                                scalar1=dl_t[ph][:, tile_col:tile_col + 1],
                                scalar2=None, op0=mybir.AluOpType.is_equal)
                            left_of_sb[sb] -= 1
                            done = left_of_sb[sb] == 0
                            nc.tensor.matmul(
                                out=pa[:],
                                lhsT=G[:, ti * elem:ti * elem + 128],
                                rhs=S[:], start=first, stop=done)
                            if joint:
                                nc.tensor.matmul(
                                    out=psum1[sb][:],
                                    lhsT=G[:, ti * elem + 128:
                                           ti * elem + 256],
                                    rhs=S[:], start=first, stop=done)
                            tile_col += 1
                    for sb in range(m * macro, min((m + 1) * macro, NSB)):
                        if sb not in psum0:
                            continue
                        wdt = _sb_width(sb)
                        pa = psum0.pop(sb)
                        stage_b(ph, sb, pa, xown, wdir, wself, biasp,
                                own_out, ag, ag_src, co_mode, wdt)
                        if joint:
                            pa1 = psum1.pop(sb)
                            nc.scalar.copy(
                                out=agg_h[:, sb * SB:sb * SB + wdt],
                                in_=pa1[:, :wdt])
                        if post_sb is not None:
                            post_sb(sb, wdt)
                    col_off += cols

            for _rep in range(reps):
                pint_own = own_chunks(f"pint{_rep}", width=2 * D)
                pint_full = full_chunks(f"pintf{_rep}", width=2 * D)
                a1a_own = own_chunks(f"a1a{_rep}")
                a1a_full = full_chunks(f"a1a{_rep}")
                a1_own = own_chunks(f"a1{_rep}")
                a1_full = full_chunks(f"a1{_rep}")
                a2a_own = own_chunks(f"a2a{_rep}")
                a2a_full = full_chunks(f"a2a{_rep}")
                a2_own = drp.tile([SHARD, D], F16, name=f"a2_o{_rep}")
                a2_ownc = [a2_own[CH_START[c]:CH_START[c] + CH_SIZE[c], :]
                           for c in range(NCHUNK)]
                p2_own = drp.tile([SHARD, D], F16, name=f"p2_o{_rep}")
                p2_ownc = [p2_own[CH_START[c]:CH_START[c] + CH_SIZE[c], :]
                           for c in range(NCHUNK)]
                sup_send = drp.tile([NCORES * blk_t * 128, D], F16,
                                    name=f"sups{_rep}")
                sup_recv = drp.tile([NCORES * blk_t * 128, D], F16,
                                    name=f"supr{_rep}")

                # stage own interleaved paper chunks (t0 halves) via SBUF,
                # two half-chunks each to bound the staging tile.
                for c in range(NCHUNK):
                    h0 = CH_SIZE[c] // 2
                    for (r0, rn) in ((0, h0), (h0, CH_SIZE[c] - h0)):
                        nflat = rn * 2 * D
                        st = stg.tile([128, 4096], F16, tag="st", name="st")
                        nc.sync.dma_start(
                            out=st[:, :nflat // 128],
                            in_=pint_in[c][r0:r0 + rn, :].rearrange(
                                "a b -> (a b)").rearrange(
                                "(p x) -> p x", p=128))
                        nc.sync.dma_start(
                            out=pint_own[c][r0:r0 + rn, :].rearrange(
                                "a b -> (a b)").rearrange(
                                "(p x) -> p x", p=128),
                            in_=st[:, :nflat // 128])

                process_phase("a2p0", author_c, xpT,
                              w_slice(0, 0), w_slice(0, 1), bias_slice(0, 0),
                              [t[:, 128:256] for t in pint_own],
                              ag=pint_full, ag_src=pint_own)
                process_phase("p2a0", [t[:] for t in pint_full], xaT,
                              w_slice(0, 2), w_slice(0, 3), bias_slice(0, 1),
                              a1a_own, ag=a1a_full, joint=(agg_h,))

                def a2a_post(sb, wdt):
                    agg_ps = psB.tile([128, 512], F32, tag="out", name="pbh")
                    deg_t = dgp.tile([2, 512], F16, tag="deg", name="degt")
                    nc.sync.dma_start(
                        out=deg_t[:, :wdt],
                        in_=deg_in["p2a0"][:, sb * SB:sb * SB + wdt])
                    nc.tensor.matmul(out=agg_ps[:, :wdt], lhsT=w_slice(1, 2),
                                     rhs=agg_h[:, sb * SB:sb * SB + wdt],
                                     start=True, stop=False)
                    nc.tensor.matmul(out=agg_ps[:, :wdt], lhsT=w_slice(1, 3),
                                     rhs=xaT[:, sb * SB:sb * SB + wdt],
                                     start=False, stop=False)
                    nc.tensor.matmul(out=agg_ps[:, :wdt],
                                     lhsT=bias_slice(1, 1),
                                     rhs=deg_t[:2, :wdt],
                                     start=False, stop=True)
                    nc.scalar.copy(
                        out=xaT[:, sb * SB:sb * SB + wdt],
                        in_=agg_ps[:, :wdt])
                    cid = transpose_writeback(xaT, sb, wdt, a2a_own)
                    if sb == CH_SB[cid][1] - 1:
                        fire_ag(cid, a2a_own, a2a_full, None)

                process_phase("co0", [t[:] for t in a1a_full], xaT,
                              w_slice(0, 4), None, bias_slice(0, 2),
                              a1_own, ag=a1_full, co_mode=True,
                              post_sb=a2a_post)

                process_phase("a2p1", [t[:] for t in a1_full], xpT,
                              w_slice(1, 0), w_slice(1, 1), bias_slice(1, 0),
                              p2_ownc)

                # sup p-side: gather local p2 rows in (dest-core, pair)
                # order, stream to sup_send, then AllToAll.
                nsupt_all = NCORES * blk_t
                chunks = [(t0, min(SUP_CHUNK, nsupt_all - t0))
                          for t0 in range(0, nsupt_all, SUP_CHUNK)]
                for (ts, ntl) in chunks:
                    nidx = ntl * 128
                    Gs = sup_pl.tile([128, SUP_CHUNK * 128], F16, tag="Gs",
                                     name="Gs")
                    nc.gpsimd.dma_gather(
                        Gs[:, :ntl * 128].rearrange("p (c e) -> p c e", e=128),
                        p2_own[:], idx_sup_p_t[:, ts * 8:(ts + ntl) * 8],
                        nidx, nidx, 128, single_packet=(nidx <= 1024),
                        queue_num=(2 * ts + 1) % 4)
                    nc.sync.dma_start(
                        out=sup_send[ts * 128:(ts + ntl) * 128, :].rearrange(
                            "(c p) f -> p c f", p=128),
                        in_=Gs[:, :ntl * 128].rearrange(
                            "p (c f) -> p c f", f=128))
                nc.gpsimd.collective_compute(
                    "AllToAll", mybir.AluOpType.bypass,
                    replica_groups=[list(range(NCORES))],
                    ins=[sup_send[:]], outs=[sup_recv[:]])

                process_phase("co1", [t[:] for t in a2a_full], xaT,
                              w_slice(1, 4), None, bias_slice(1, 2),
                              a2_ownc, co_mode=True)

                for (ts, ntl) in chunks:
                    nidx = ntl * 128
                    Gp = sup_pl.tile([128, SUP_CHUNK * 128], F16, tag="Gp",
                                     name="Gp")
                    nc.sync.dma_start(
                        out=Gp[:, :ntl * 128].rearrange(
                            "p (c f) -> p c f", f=128),
                        in_=sup_recv[ts * 128:(ts + ntl) * 128, :].rearrange(
                            "(c p) f -> p c f", p=128))
                    Ga = sup_pl.tile([128, SUP_CHUNK * 128], F16, tag="Ga",
                                     name="Ga")
                    nc.gpsimd.dma_gather(
                        Ga[:, :ntl * 128].rearrange("p (c e) -> p c e", e=128),
                        a2_own[:], idx_sup_a_t[:, ts * 8:(ts + ntl) * 8],
                        nidx, nidx, 128, single_packet=(nidx <= 1024),
                        queue_num=(2 * ts) % 4)
                    for t in range(ntl):
                        prod = sup_pl.tile([128, 128], F32, tag="prod",
                                           name="prod")
                        nc.vector.tensor_tensor(
                            out=prod[:],
                            in0=Ga[:, t * 128:t * 128 + 128],
                            in1=Gp[:, t * 128:t * 128 + 128],
                            op=mybir.AluOpType.mult)
                        nc.vector.reduce_sum(
                            out=out_sb[:, ts + t:ts + t + 1], in_=prod[:],
                            axis=mybir.AxisListType.X)
            nc.sync.dma_start(out=out_sup[:], in_=out_sb[:])
    nc.compile()
    return nc


# ---------------------------------------------------------------- interface

_CACHE = {}


def _preprocess(inputs):
    xa = np.asarray(inputs["x_author"], np.float32).astype(np.float16)
    xp = np.asarray(inputs["x_paper"], np.float32)[:N_PAPER_ACT].astype(
        np.float16)
    ei = np.asarray(inputs["edge_index"], np.int64)
    ci = np.asarray(inputs["coauthor_edge_index"], np.int64)
    si = np.asarray(inputs["supervision_edge_index"], np.int64)

    sup_author = np.zeros(N_AUTHOR, bool)
    sup_author[si[0]] = True
    sup_paper = np.zeros(N_PAPER_ACT, bool)
    sup_paper[si[1]] = True
    a2p_l1 = ei[:, sup_paper[ei[1]]]
    co_l1 = ci[:, sup_author[ci[1]]]

    prep = {}
    prep["a2p0"] = _prep_direction(ei[0], ei[1])
    prep["p2a0"] = _prep_direction(ei[1], ei[0], macro=MACRO_J)
    prep["co0"] = _prep_direction(ci[0], ci[1])
    prep["a2p1"] = _prep_direction(a2p_l1[0], a2p_l1[1])
    prep["co1"] = _prep_direction(co_l1[0], co_l1[1])
    sup_packs, sup_blk_t = _prep_sup(si[0], si[1])

    meta = {"sup_blk_t": sup_blk_t}
    for ph in PHASES:
        meta[ph] = prep[ph][3]
        meta[f"ntiles_{ph}"] = prep[ph][1][0].shape[1]

    ws, bs = [], []
    for l in range(N_LAYERS):
        for nm in ["W_a2p", "W_pself", "W_p2a", "W_aself", "W_co"]:
            ws.append(np.asarray(inputs[nm], np.float32)[l].T.astype(
                np.float16))
        for pair in [("b_a2p", "b_pself"), ("b_p2a", "b_aself"),
                     ("b_co", None)]:
            r0 = np.asarray(inputs[pair[0]], np.float32)[l]
            r1 = (np.asarray(inputs[pair[1]], np.float32)[l]
                  if pair[1] else np.zeros(D, np.float32))
            bs.append(np.stack([r0, r1]).astype(np.float16))
    w_cat = np.concatenate(ws, axis=1)
    bias_cat = np.concatenate(bs, axis=1)
    iota = np.broadcast_to(np.arange(512, dtype=np.float16), (128, 512)).copy()
    ident = np.eye(128, dtype=np.float16)

    def chunkify(x):
        out = []
        for c in range(NCHUNK):
            rows = [x[k * SHARD + CH_START[c]:
                      k * SHARD + CH_START[c] + CH_SIZE[c]]
                    for k in range(NCORES)]
            out.append(np.concatenate(rows, axis=0))
        return out

    author_ch = chunkify(xa)

    in_maps = []
    for k in range(NCORES):
        im = {
            "xaT0": xa[k * SHARD:(k + 1) * SHARD].T.copy(),
            "xpT0": xp[k * SHARD:(k + 1) * SHARD].T.copy(),
            "w_cat": w_cat, "bias_cat": bias_cat,
            "iota512": iota, "identity": ident,
            "idx_sup_a": sup_packs[k][0],
            "idx_sup_p": sup_packs[k][1],
        }
        for c in range(NCHUNK):
            im[f"author_c{c}"] = author_ch[c]
            pint = np.zeros((CH_SIZE[c], 2 * D), np.float16)
            pint[:, :D] = xp[k * SHARD + CH_START[c]:
                             k * SHARD + CH_START[c] + CH_SIZE[c]]
            im[f"pint_c{c}"] = pint
        for ph in PHASES:
            idxs, dls, degs, _ = prep[ph]
            im[f"idx_{ph}"] = idxs[k]
            im[f"dl_{ph}"] = dls[k]
            im[f"deg_{ph}"] = degs[k]
        in_maps.append(im)
    recon = [p[2] for p in sup_packs]
    return in_maps, meta, recon, si


def _postprocess(results, meta, recon):
    blk_t = int(meta["sup_blk_t"])
    blk = blk_t * 128
    out = np.zeros(100000, np.float32)
    for k in range(NCORES):
        o = results[k]["out_sup"]
        for s in range(NCORES):
            pos = recon[k][s]
            n = len(pos)
            vals = o[:, s * blk_t:(s + 1) * blk_t].T.reshape(-1)[:n]
            out[pos] = vals
    return out


def kernel(**inputs):
    in_maps, meta, recon, _si = _preprocess(inputs)
    key = "prog"
    if key not in _CACHE:
        _CACHE[key] = _build_program(meta)
    nc = _CACHE[key]
    res = run_bass_kernel_spmd(nc, in_maps, core_ids=list(range(NCORES)))
    return _postprocess(res.results, meta, recon)


# revision 29
# speedup vs baseline: 1.0107x; 1.0107x over previous
"""BipartiteGCN Trainium2 kernel v5 (8 NeuronCores, Bass/Tile).

v3 architecture (256B fp16 rows, chunked tables + chunked AllGathers,
width-512 one-hot aggregation, layer-1 a2p/co supervision filtering,
joint p2a gather) plus:
  - asymmetric source chunks (4096/4096/3584/724 rows): the last AllGather
    of every phase is tiny, shortening the phase-boundary serial tail.
  - the layer-1 p2a stage-B pass (a2a) fused into co0's stage-B per block,
    so the a2a AllGathers fire spread across co0 instead of bunched after.
  - supervision via AllToAll: p2 rows for sup pairs gathered locally on
    the paper-owner core, AllToAll'ed to the author-owner core (no p2
    AllGathers; recv side is plain contiguous loads).
  - scalar-engine psum evacuation, fused multiply+reduce for sup dots,
    deeper gather/one-hot pools.
"""

import numpy as np

import concourse.bacc as bacc
import concourse.mybir as mybir
import concourse.tile as tile
from concourse.bass_utils import run_bass_kernel_spmd

F32 = mybir.dt.float32
F16 = mybir.dt.float16
I16 = mybir.dt.int16

NCORES = 8
D = 128
N_AUTHOR = 100000
N_PAPER_ACT = 100000
SHARD = N_AUTHOR // NCORES      # 12500
SB = 512
NSB = (SHARD + SB - 1) // SB    # 25
MACRO = 5
MACRO_J = 2                     # joint phase: 2 PSUM banks per sb
NCHUNK = 5
CH_START = [0, 4096, 7168, 9728, 11776]
CH_SIZE = [4096, 3072, 2560, 2048, SHARD - 11776]
CH_SB = [(0, 8), (8, 14), (14, 19), (19, 23), (23, 25)]
PAD_DST = 5000.0
N_LAYERS = 2
SUP_CHUNK = 8


def _sb_width(sb):
    return min(SB, SHARD - sb * SB)


def _chunk_of_local(r):
    return np.searchsorted(np.array(CH_START[1:] + [SHARD]), r, side="right")


def _chunk_row(node):
    k = node // SHARD
    r = node - k * SHARD
    c = _chunk_of_local(r)
    sz = np.array(CH_SIZE)[c]
    st = np.array(CH_START)[c]
    return c, k * sz + (r - st)


# ---------------------------------------------------------------- host prep

def _wrap_idx(idx):
    n = len(idx)
    w = np.zeros((128, n // 16), np.int16)
    base = idx.astype(np.int16).reshape(n // 16, 16).T
    for g in range(8):
        w[16 * g:16 * g + 16, :] = base
    return w


def _build_tiles_one_core(src, dst_local):
    sb_id = dst_local // SB
    off = dst_local - sb_id * SB
    c, crow = _chunk_row(src)
    tiles = {}
    nt = np.zeros((NSB, NCHUNK), np.int64)
    order = np.lexsort((off, c, sb_id))
    sb_s, c_s = sb_id[order], c[order]
    off_s, crow_s = off[order], crow[order]
    key = sb_s * NCHUNK + c_s
    bounds = np.flatnonzero(np.diff(key)) + 1
    starts = np.concatenate(([0], bounds))
    ends = np.concatenate((bounds, [len(key)]))
    for s, e in zip(starts, ends):
        sb, cc = int(sb_s[s]), int(c_s[s])
        r = crow_s[s:e]
        o = off_s[s:e]
        group = [(r[i:i + 128], o[i:i + 128]) for i in range(0, e - s, 128)]
        tiles[(sb, cc)] = group
        nt[sb, cc] = len(group)
    return nt, tiles


def _emit_direction(all_tiles, global_nt, macro):
    nmacro = (NSB + macro - 1) // macro
    idx_stream = []
    dstloc_cols = []
    meta = []
    first_seen = set()
    last_pos = {}
    for sb in range(NSB):
        tot = int(global_nt[sb].sum())
        assert tot > 0
        cnt = 0
        for c in range(NCHUNK):
            for t in range(int(global_nt[sb, c])):
                cnt += 1
                if cnt == tot:
                    last_pos[sb] = (c, t)
    for m in range(nmacro):
        sbs = range(m * macro, min((m + 1) * macro, NSB))
        for c in range(NCHUNK):
            tl = []
            for sb in sbs:
                group = all_tiles.get((sb, c), [])
                for t in range(int(global_nt[sb, c])):
                    if t < len(group):
                        r, o = group[t]
                    else:
                        r = np.zeros(0, np.int64)
                        o = np.zeros(0, np.int64)
                    n = len(r)
                    first = sb not in first_seen
                    if first:
                        first_seen.add(sb)
                    last = last_pos[sb] == (c, t)
                    src128 = np.zeros(128, np.int64)
                    dl128 = np.full(128, PAD_DST, np.float32)
                    src128[:n] = r
                    dl128[:n] = o[:n]
                    idx_stream.append(src128)
                    dstloc_cols.append(dl128)
                    tl.append((sb, first, last))
            meta.append(tl)
    ntiles = len(idx_stream)
    idx_flat = np.concatenate(idx_stream) if ntiles else np.zeros(0, np.int64)
    dstloc = (np.stack(dstloc_cols, axis=1) if ntiles
              else np.zeros((128, 0), np.float32))
    return _wrap_idx(idx_flat), dstloc.astype(np.float32), meta


def _prep_direction(src_all, dst_all, macro=MACRO, ncores=NCORES):
    owner = dst_all // SHARD
    per_core = []
    nts = []
    for k in range(ncores):
        m = owner == k
        nt, tiles = _build_tiles_one_core(src_all[m], dst_all[m] - k * SHARD)
        nts.append(nt)
        per_core.append(tiles)
    global_nt = np.maximum.reduce(nts)
    global_nt[:, 0] = np.maximum(global_nt[:, 0], 1)
    idxs, dstlocs, metas = [], [], []
    for k in range(ncores):
        iw, dl, meta = _emit_direction(per_core[k], global_nt, macro)
        idxs.append(iw)
        dstlocs.append(dl)
        metas.append(meta)
    degs = []
    for k in range(ncores):
        m = owner == k
        deg = np.bincount(dst_all[m] - k * SHARD, minlength=SHARD)
        degs.append(np.stack([deg, np.ones(SHARD)]).astype(np.float16))
    return idxs, dstlocs, degs, metas[0]


def _prep_sup(sup_a, sup_p, ncores=NCORES):
    """A2A layout: paper-owner kp gathers p2 rows for its pairs grouped by
    author-owner ka; AllToAll ships block (kp -> ka); author-owner computes
    products with locally gathered a2 rows in the same pair order."""
    kp = sup_p // SHARD
    ka = sup_a // SHARD
    a_loc = sup_a - ka * SHARD
    p_loc = sup_p - kp * SHARD
    cnt = np.zeros((ncores, ncores), np.int64)
    np.add.at(cnt, (kp, ka), 1)
    blk_t = int((cnt.max() + 127) // 128)
    blk = blk_t * 128
    p_send = np.zeros((ncores, ncores * blk), np.int64)
    a_recv = np.zeros((ncores, ncores * blk), np.int64)
    pos = [[None] * ncores for _ in range(ncores)]
    for s in range(ncores):
        for r in range(ncores):
            m = np.flatnonzero((kp == s) & (ka == r))
            sub = m[np.lexsort((m, p_loc[m], a_loc[m]))]
            nb = len(sub)
            p_send[s, r * blk:r * blk + nb] = p_loc[sub]
            a_recv[r, s * blk:s * blk + nb] = a_loc[sub]
            pos[r][s] = sub
    packs = []
    for k in range(ncores):
        packs.append((_wrap_idx(a_recv[k]), _wrap_idx(p_send[k]), pos[k]))
    return packs, blk_t


# ------------------------------------------------------------- program build

PHASES = ["a2p0", "p2a0", "co0", "a2p1", "co1"]


def _build_program(meta, reps=1):
    nc = bacc.Bacc("TRN2", target_bir_lowering=False, debug=False,
                   enable_asserts=False, num_devices=NCORES,
                   num_swdge_queues=4)

    def din(name, shape, dt=F16):
        return nc.dram_tensor(name, shape, dt, kind="ExternalInput").ap()

    author_c = [din(f"author_c{c}", [8 * CH_SIZE[c], D]) for c in range(NCHUNK)]
    # own interleaved paper chunks: [:, :128] = own paper_t0, [:, 128:] = 0
    pint_in = [din(f"pint_c{c}", [CH_SIZE[c], 2 * D]) for c in range(NCHUNK)]
    xaT0 = din("xaT0", [128, SHARD])
    xpT0 = din("xpT0", [128, SHARD])
    w_cat = din("w_cat", [128, 128 * 10])
    bias_cat = din("bias_cat", [2, 128 * 6])
    iota_in = din("iota512", [128, 512], F16)
    ident_in = din("identity", [128, 128])
    idx_in, dl_in, deg_in = {}, {}, {}
    for ph in PHASES:
        ntp = meta[f"ntiles_{ph}"]
        idx_in[ph] = din(f"idx_{ph}", [128, ntp * 8], I16)
        dl_in[ph] = din(f"dl_{ph}", [128, ntp], F32)
        deg_in[ph] = din(f"deg_{ph}", [2, SHARD])
    blk_t = int(meta["sup_blk_t"])
    nsup = NCORES * blk_t * 128
    nsupt = nsup // 128                      # total product tiles
    idx_sup_a = din("idx_sup_a", [128, nsup // 16], I16)
    idx_sup_p = din("idx_sup_p", [128, nsup // 16], I16)
    out_sup = nc.dram_tensor("out_sup", [128, nsupt], F32,
                             kind="ExternalOutput").ap()

    # sizing: G pool bytes and idx columns per macro
    g_bytes_max = 1
    idx_cols_max = 16
    for ph in PHASES:
        mm = meta[ph]
        nmacro = len(mm) // NCHUNK
        elem = 256 if ph == "p2a0" else 128
        for tl in mm:
            g_bytes_max = max(g_bytes_max, len(tl) * elem * 2)
        for m in range(nmacro):
            cols = sum(len(mm[m * NCHUNK + c]) for c in range(NCHUNK)) * 8
            idx_cols_max = max(idx_cols_max, cols)
    g_cols = g_bytes_max // 2   # fp16 elems per partition

    with tile.TileContext(nc) as tc:
        with tc.tile_pool(name="persist", bufs=1) as pp, \
             tc.tile_pool(name="gat", bufs=6) as gp, \
             tc.tile_pool(name="oneh", bufs=10) as sp, \
             tc.tile_pool(name="stageb", bufs=3) as bp, \
             tc.tile_pool(name="degp", bufs=4) as dgp, \
             tc.tile_pool(name="idxp", bufs=3) as ixp, \
             tc.tile_pool(name="supp", bufs=3) as sup_pl, \
             tc.tile_pool(name="stg", bufs=1) as stg, \
             tc.tile_pool(name="psA", bufs=5, space="PSUM") as psA, \
             tc.tile_pool(name="psB", bufs=2, space="PSUM") as psB, \
             tc.tile_pool(name="psT", bufs=1, space="PSUM") as psT, \
             tc.tile_pool(name="dram", bufs=1, space="DRAM") as drp:

            xaT = pp.tile([128, SHARD], F16, name="xaT")
            xpT = pp.tile([128, SHARD], F16, name="xpT")
            agg_h = pp.tile([128, SHARD], F16, name="agg_h")
            iota = pp.tile([128, 512], F16, name="iota")
            ident = pp.tile([128, 128], F16, name="ident")
            w_t = pp.tile([128, 128 * 10], F16, name="w_t")
            bias_t = pp.tile([2, 128 * 6], F16, name="bias_t")
            dl_t = {p: pp.tile([128, meta[f"ntiles_{p}"]], F32, name=f"dl_{p}")
                    for p in PHASES}
            out_sb = pp.tile([128, nsupt], F32, name="out_sb")

            nc.sync.dma_start(out=xaT[:], in_=xaT0[:])
            nc.sync.dma_start(out=xpT[:], in_=xpT0[:])
            nc.sync.dma_start(out=iota[:], in_=iota_in[:])
            nc.sync.dma_start(out=ident[:], in_=ident_in[:])
            nc.sync.dma_start(out=w_t[:], in_=w_cat[:])
            nc.sync.dma_start(out=bias_t[:], in_=bias_cat[:])
            for p in PHASES:
                nc.sync.dma_start(out=dl_t[p][:], in_=dl_in[p][:])
            idx_sup_a_t = pp.tile([128, nsup // 16], I16, name="supa")
            idx_sup_p_t = pp.tile([128, nsup // 16], I16, name="supb")
            nc.sync.dma_start(out=idx_sup_a_t[:], in_=idx_sup_a[:])
            nc.sync.dma_start(out=idx_sup_p_t[:], in_=idx_sup_p[:])

            def own_chunks(name, width=D):
                return [drp.tile([CH_SIZE[c], width], F16, name=f"{name}_o{c}")
                        for c in range(NCHUNK)]

            def full_chunks(name, width=D):
                return [drp.tile([8 * CH_SIZE[c], width], F16,
                                 addr_space="Shared", name=f"{name}_f{c}")
                        for c in range(NCHUNK)]

            def w_slice(l, slot):
                o = (l * 5 + slot) * 128
                return w_t[:, o:o + 128]

            def bias_slice(l, ph):
                o = (l * 3 + ph) * 128
                return bias_t[:, o:o + 128]

            def transpose_writeback(xown, sb, wdt, own_out):
                pt = psT.tile([128, 512], F16, tag="tr", name="pt")
                nch = (wdt + 127) // 128
                for j in range(nch):
                    cw = min(128, wdt - j * 128)
                    nc.tensor.matmul(
                        out=pt[:cw, j * 128:j * 128 + 128],
                        lhsT=xown[:, sb * SB + j * 128:sb * SB + j * 128 + cw],
                        rhs=ident[:], is_transpose=True,
                        start=(j == 0), stop=(j == nch - 1))
                rm = bp.tile([128, 512], F16, tag="rm", name="rm")
                nc.scalar.copy(out=rm[:, :nch * 128], in_=pt[:, :nch * 128])
                cid = int(_chunk_of_local(sb * SB))
                row0 = sb * SB - CH_START[cid]
                for j in range(nch):
                    cw = min(128, wdt - j * 128)
                    nc.sync.dma_start(
                        out=own_out[cid][row0 + j * 128:row0 + j * 128 + cw,
                                         0:128],
                        in_=rm[:cw, j * 128:j * 128 + 128])
                return cid

            def fire_ag(cid, own_out, ag, ag_src):
                src = ag_src[cid] if ag_src is not None else own_out[cid]
                nc.gpsimd.collective_compute(
                    "AllGather", mybir.AluOpType.bypass,
                    replica_groups=[list(range(NCORES))],
                    ins=[src[:]], outs=[ag[cid][:]])

            def stage_b(ph_deg, sb, pa, xown, wdir, wself, biasp, own_out,
                        ag, ag_src, co_mode, wdt=None):
                """Shared stage-B: transform+bias, xown update, transpose,
                row-major writeback, chunk AG firing."""
                wdt = wdt or _sb_width(sb)
                agg_sb = bp.tile([128, 512], F16, tag="aggsb", name="aggsb")
                nc.scalar.copy(out=agg_sb[:, :wdt], in_=pa[:, :wdt])
                deg_t = dgp.tile([2, 512], F16, tag="deg", name="degt")
                nc.sync.dma_start(
                    out=deg_t[:, :wdt],
                    in_=deg_in[ph_deg][:, sb * SB:sb * SB + wdt])
                pb = psB.tile([128, 512], F32, tag="out", name="pb")
                nc.tensor.matmul(out=pb[:, :wdt], lhsT=wdir,
                                 rhs=agg_sb[:, :wdt], start=True, stop=False)
                if not co_mode:
                    nc.tensor.matmul(
                        out=pb[:, :wdt], lhsT=wself,
                        rhs=xown[:, sb * SB:sb * SB + wdt],
                        start=False, stop=False)
                nc.tensor.matmul(out=pb[:, :wdt], lhsT=biasp,
                                 rhs=deg_t[:2, :wdt], start=False, stop=True)
                if co_mode:
                    nc.vector.tensor_tensor(
                        out=xown[:, sb * SB:sb * SB + wdt],
                        in0=pb[:, :wdt],
                        in1=xown[:, sb * SB:sb * SB + wdt],
                        op=mybir.AluOpType.add)
                else:
                    nc.scalar.copy(
                        out=xown[:, sb * SB:sb * SB + wdt], in_=pb[:, :wdt])
                cid = transpose_writeback(xown, sb, wdt, own_out)
                if ag is not None and sb == CH_SB[cid][1] - 1:
                    fire_ag(cid, own_out, ag, ag_src)

            def process_phase(ph, src_tbls, xown, wdir, wself, biasp,
                              own_out, ag=None, ag_src=None, co_mode=False,
                              joint=None, post_sb=None):
                """joint: None, or (agg_hold_tile,) for the p2a joint phase
                (elem=256, second matmul per tile into a held aggregate).
                post_sb: optional callback(sb, wdt) run after stage_b(sb)."""
                mm = meta[ph]
                nmacro = len(mm) // NCHUNK
                macro = (NSB + nmacro - 1) // nmacro
                elem = 256 if joint else 128
                tile_col = 0
                psum0, psum1 = {}, {}
                left_of_sb = {sb: 0 for sb in range(NSB)}
                for tl in mm:
                    for (sb, _f, _l) in tl:
                        left_of_sb[sb] += 1
                col_off = 0
                for m in range(nmacro):
                    cols = sum(len(mm[m * NCHUNK + c])
                               for c in range(NCHUNK)) * 8
                    if cols == 0:
                        continue
                    idx_t = ixp.tile([128, idx_cols_max], I16, tag="idx",
                                     name="idxt")
                    nc.sync.dma_start(
                        out=idx_t[:, :cols],
                        in_=idx_in[ph][:, col_off:col_off + cols])
                    mac_off = 0
                    for c in range(NCHUNK):
                        tl = mm[m * NCHUNK + c]
                        ntl = len(tl)
                        if ntl == 0:
                            continue
                        nidx = ntl * 128
                        G = gp.tile([128, g_cols], F16, tag="G", name="G")
                        nc.gpsimd.dma_gather(
                            G[:, :ntl * elem].rearrange(
                                "p (c e) -> p c e", e=elem),
                            src_tbls[c][:, :],
                            idx_t[:, mac_off:mac_off + ntl * 8],
                            nidx, nidx, elem,
                            single_packet=(nidx <= 1024),
                            queue_num=c % 4)
                        mac_off += ntl * 8
                        for ti, (sb, first, _last) in enumerate(tl):
                            if sb not in psum0:
                                psum0[sb] = psA.tile([128, 512], F32,
                                                     tag="agg", name="agg")
                                if joint:
                                    psum1[sb] = psA.tile(
                                        [128, 512], F32, tag="agg",
                                        name="agg")
                            pa = psum0[sb]
                            S = sp.tile([128, 512], F16, tag="S", name="S")
                            nc.vector.tensor_scalar(
                                out=S[:], in0=iota[:],
                                scalar1=dl_t[ph][:, tile_col:tile_col + 1],
                                scalar2=None, op0=mybir.AluOpType.is_equal)
                            left_of_sb[sb] -= 1
                            done = left_of_sb[sb] == 0
                            nc.tensor.matmul(
                                out=pa[:],
                                lhsT=G[:, ti * elem:ti * elem + 128],
                                rhs=S[:], start=first, stop=done)
                            if joint:
                                nc.tensor.matmul(
                                    out=psum1[sb][:],
                                    lhsT=G[:, ti * elem + 128:
                                           ti * elem + 256],
                                    rhs=S[:], start=first, stop=done)
                            tile_col += 1
                    for sb in range(m * macro, min((m + 1) * macro, NSB)):
                        if sb not in psum0:
                            continue
                        wdt = _sb_width(sb)
                        pa = psum0.pop(sb)
                        stage_b(ph, sb, pa, xown, wdir, wself, biasp,
                                own_out, ag, ag_src, co_mode, wdt)
                        if joint:
                            pa1 = psum1.pop(sb)
                            nc.scalar.copy(
                                out=agg_h[:, sb * SB:sb * SB + wdt],
                                in_=pa1[:, :wdt])
                        if post_sb is not None:
                            post_sb(sb, wdt)
                    col_off += cols

            for _rep in range(reps):
                pint_own = own_chunks(f"pint{_rep}", width=2 * D)
                pint_full = full_chunks(f"pintf{_rep}", width=2 * D)
                a1a_own = own_chunks(f"a1a{_rep}")
                a1a_full = full_chunks(f"a1a{_rep}")
                a1_own = own_chunks(f"a1{_rep}")
                a1_full = full_chunks(f"a1{_rep}")
                a2a_own = own_chunks(f"a2a{_rep}")
                a2a_full = full_chunks(f"a2a{_rep}")
                a2_own = drp.tile([SHARD, D], F16, name=f"a2_o{_rep}")
                a2_ownc = [a2_own[CH_START[c]:CH_START[c] + CH_SIZE[c], :]
                           for c in range(NCHUNK)]
                p2_own = drp.tile([SHARD, D], F16, name=f"p2_o{_rep}")
                p2_ownc = [p2_own[CH_START[c]:CH_START[c] + CH_SIZE[c], :]
                           for c in range(NCHUNK)]
                sup_send = drp.tile([NCORES * blk_t * 128, D], F16,
                                    name=f"sups{_rep}")
                sup_recv = drp.tile([NCORES * blk_t * 128, D], F16,
                                    name=f"supr{_rep}")

                # stage own interleaved paper chunks (t0 halves) via SBUF,
                # two half-chunks each to bound the staging tile.
                for c in range(NCHUNK):
                    h0 = CH_SIZE[c] // 2
                    for (r0, rn) in ((0, h0), (h0, CH_SIZE[c] - h0)):
                        nflat = rn * 2 * D
                        st = stg.tile([128, 4096], F16, tag="st", name="st")
                        nc.sync.dma_start(
                            out=st[:, :nflat // 128],
                            in_=pint_in[c][r0:r0 + rn, :].rearrange(
                                "a b -> (a b)").rearrange(
                                "(p x) -> p x", p=128))
                        nc.sync.dma_start(
                            out=pint_own[c][r0:r0 + rn, :].rearrange(
                                "a b -> (a b)").rearrange(
                                "(p x) -> p x", p=128),
                            in_=st[:, :nflat // 128])

                process_phase("a2p0", author_c, xpT,
                              w_slice(0, 0), w_slice(0, 1), bias_slice(0, 0),
                              [t[:, 128:256] for t in pint_own],
                              ag=pint_full, ag_src=pint_own)
                process_phase("p2a0", [t[:] for t in pint_full], xaT,
                              w_slice(0, 2), w_slice(0, 3), bias_slice(0, 1),
                              a1a_own, ag=a1a_full, joint=(agg_h,))

                process_phase("co0", [t[:] for t in a1a_full], xaT,
                              w_slice(0, 4), None, bias_slice(0, 2),
                              a1_own, ag=a1_full, co_mode=True)

                # gather-free stage B for a2a (layer-1 p2a, held aggregate)
                for sb in range(NSB):
                    wdt = _sb_width(sb)
                    agg_ps = psB.tile([128, 512], F32, tag="out", name="pbh")
                    deg_t = dgp.tile([2, 512], F16, tag="deg", name="degt")
                    nc.sync.dma_start(
                        out=deg_t[:, :wdt],
                        in_=deg_in["p2a0"][:, sb * SB:sb * SB + wdt])
                    nc.tensor.matmul(out=agg_ps[:, :wdt], lhsT=w_slice(1, 2),
                                     rhs=agg_h[:, sb * SB:sb * SB + wdt],
                                     start=True, stop=False)
                    nc.tensor.matmul(out=agg_ps[:, :wdt], lhsT=w_slice(1, 3),
                                     rhs=xaT[:, sb * SB:sb * SB + wdt],
                                     start=False, stop=False)
                    nc.tensor.matmul(out=agg_ps[:, :wdt],
                                     lhsT=bias_slice(1, 1),
                                     rhs=deg_t[:2, :wdt],
                                     start=False, stop=True)
                    nc.scalar.copy(
                        out=xaT[:, sb * SB:sb * SB + wdt],
                        in_=agg_ps[:, :wdt])
                    cid = transpose_writeback(xaT, sb, wdt, a2a_own)
                    if sb == CH_SB[cid][1] - 1:
                        fire_ag(cid, a2a_own, a2a_full, None)

                process_phase("a2p1", [t[:] for t in a1_full], xpT,
                              w_slice(1, 0), w_slice(1, 1), bias_slice(1, 0),
                              p2_ownc)

                # sup p-side: gather local p2 rows in (dest-core, pair)
                # order, stream to sup_send, then AllToAll.
                nsupt_all = NCORES * blk_t
                chunks = [(t0, min(SUP_CHUNK, nsupt_all - t0))
                          for t0 in range(0, nsupt_all, SUP_CHUNK)]
                for (ts, ntl) in chunks:
                    nidx = ntl * 128
                    Gs = sup_pl.tile([128, SUP_CHUNK * 128], F16, tag="Gs",
                                     name="Gs")
                    nc.gpsimd.dma_gather(
                        Gs[:, :ntl * 128].rearrange("p (c e) -> p c e", e=128),
                        p2_own[:], idx_sup_p_t[:, ts * 8:(ts + ntl) * 8],
                        nidx, nidx, 128, single_packet=(nidx <= 1024),
                        queue_num=(2 * ts + 1) % 4)
                    nc.sync.dma_start(
                        out=sup_send[ts * 128:(ts + ntl) * 128, :].rearrange(
                            "(c p) f -> p c f", p=128),
                        in_=Gs[:, :ntl * 128].rearrange(
                            "p (c f) -> p c f", f=128))
                nc.gpsimd.collective_compute(
                    "AllToAll", mybir.AluOpType.bypass,
                    replica_groups=[list(range(NCORES))],
                    ins=[sup_send[:]], outs=[sup_recv[:]])

                process_phase("co1", [t[:] for t in a2a_full], xaT,
                              w_slice(1, 4), None, bias_slice(1, 2),
                              a2_ownc, co_mode=True)

                for (ts, ntl) in chunks:
                    nidx = ntl * 128
                    Gp = sup_pl.tile([128, SUP_CHUNK * 128], F16, tag="Gp",
                                     name="Gp")
                    nc.sync.dma_start(
                        out=Gp[:, :ntl * 128].rearrange(
                            "p (c f) -> p c f", f=128),
                        in_=sup_recv[ts * 128:(ts + ntl) * 128, :].rearrange(
                            "(c p) f -> p c f", p=128))
                    Ga = sup_pl.tile([128, SUP_CHUNK * 128], F16, tag="Ga",
                                     name="Ga")
                    nc.gpsimd.dma_gather(
                        Ga[:, :ntl * 128].rearrange("p (c e) -> p c e", e=128),
                        a2_own[:], idx_sup_a_t[:, ts * 8:(ts + ntl) * 8],
                        nidx, nidx, 128, single_packet=(nidx <= 1024),
                        queue_num=(2 * ts) % 4)
                    for t in range(ntl):
                        prod = sup_pl.tile([128, 128], F32, tag="prod",
                                           name="prod")
                        nc.vector.tensor_tensor(
                            out=prod[:],
                            in0=Ga[:, t * 128:t * 128 + 128],
                            in1=Gp[:, t * 128:t * 128 + 128],
                            op=mybir.AluOpType.mult)
                        nc.vector.reduce_sum(
                            out=out_sb[:, ts + t:ts + t + 1], in_=prod[:],
                            axis=mybir.AxisListType.X)
            nc.sync.dma_start(out=out_sup[:], in_=out_sb[:])
    nc.compile()
    return nc


# ---------------------------------------------------------------- interface

_CACHE = {}


def _preprocess(inputs):
    xa = np.asarray(inputs["x_author"], np.float32).astype(np.float16)
    xp = np.asarray(inputs["x_paper"], np.float32)[:N_PAPER_ACT].astype(
        np.float16)
    ei = np.asarray(inputs["edge_index"], np.int64)
    ci = np.asarray(inputs["coauthor_edge_index"], np.int64)
    si = np.asarray(inputs["supervision_edge_index"], np.int64)

    sup_author = np.zeros(N_AUTHOR, bool)
    sup_author[si[0]] = True
    sup_paper = np.zeros(N_PAPER_ACT, bool)
    sup_paper[si[1]] = True
    a2p_l1 = ei[:, sup_paper[ei[1]]]
    co_l1 = ci[:, sup_author[ci[1]]]

    prep = {}
    prep["a2p0"] = _prep_direction(ei[0], ei[1])
    prep["p2a0"] = _prep_direction(ei[1], ei[0], macro=MACRO_J)
    prep["co0"] = _prep_direction(ci[0], ci[1])
    prep["a2p1"] = _prep_direction(a2p_l1[0], a2p_l1[1])
    prep["co1"] = _prep_direction(co_l1[0], co_l1[1])
    sup_packs, sup_blk_t = _prep_sup(si[0], si[1])

    meta = {"sup_blk_t": sup_blk_t}
    for ph in PHASES:
        meta[ph] = prep[ph][3]
        meta[f"ntiles_{ph}"] = prep[ph][1][0].shape[1]

    ws, bs = [], []
    for l in range(N_LAYERS):
        for nm in ["W_a2p", "W_pself", "W_p2a", "W_aself", "W_co"]:
            ws.append(np.asarray(inputs[nm], np.float32)[l].T.astype(
                np.float16))
        for pair in [("b_a2p", "b_pself"), ("b_p2a", "b_aself"),
                     ("b_co", None)]:
            r0 = np.asarray(inputs[pair[0]], np.float32)[l]
            r1 = (np.asarray(inputs[pair[1]], np.float32)[l]
                  if pair[1] else np.zeros(D, np.float32))
            bs.append(np.stack([r0, r1]).astype(np.float16))
    w_cat = np.concatenate(ws, axis=1)
    bias_cat = np.concatenate(bs, axis=1)
    iota = np.broadcast_to(np.arange(512, dtype=np.float16), (128, 512)).copy()
    ident = np.eye(128, dtype=np.float16)

    def chunkify(x):
        out = []
        for c in range(NCHUNK):
            rows = [x[k * SHARD + CH_START[c]:
                      k * SHARD + CH_START[c] + CH_SIZE[c]]
                    for k in range(NCORES)]
            out.append(np.concatenate(rows, axis=0))
        return out

    author_ch = chunkify(xa)

    in_maps = []
    for k in range(NCORES):
        im = {
            "xaT0": xa[k * SHARD:(k + 1) * SHARD].T.copy(),
            "xpT0": xp[k * SHARD:(k + 1) * SHARD].T.copy(),
            "w_cat": w_cat, "bias_cat": bias_cat,
            "iota512": iota, "identity": ident,
            "idx_sup_a": sup_packs[k][0],
            "idx_sup_p": sup_packs[k][1],
        }
        for c in range(NCHUNK):
            im[f"author_c{c}"] = author_ch[c]
            pint = np.zeros((CH_SIZE[c], 2 * D), np.float16)
            pint[:, :D] = xp[k * SHARD + CH_START[c]:
                             k * SHARD + CH_START[c] + CH_SIZE[c]]
            im[f"pint_c{c}"] = pint
        for ph in PHASES:
            idxs, dls, degs, _ = prep[ph]
            im[f"idx_{ph}"] = idxs[k]
            im[f"dl_{ph}"] = dls[k]
            im[f"deg_{ph}"] = degs[k]
        in_maps.append(im)
    recon = [p[2] for p in sup_packs]
    return in_maps, meta, recon, si


def _postprocess(results, meta, recon):
    blk_t = int(meta["sup_blk_t"])
    blk = blk_t * 128
    out = np.zeros(100000, np.float32)
    for k in range(NCORES):
        o = results[k]["out_sup"]
        for s in range(NCORES):
            pos = recon[k][s]
            n = len(pos)
            vals = o[:, s * blk_t:(s + 1) * blk_t].T.reshape(-1)[:n]
            out[pos] = vals
    return out


def kernel(**inputs):
    in_maps, meta, recon, _si = _preprocess(inputs)
    key = "prog"
    if key not in _CACHE:
        _CACHE[key] = _build_program(meta)
    nc = _CACHE[key]
    res = run_bass_kernel_spmd(nc, in_maps, core_ids=list(range(NCORES)))
    return _postprocess(res.results, meta, recon)
